# revision 1
# baseline (speedup 1.0000x reference)
"""Multi-head attention (b=2, s=2048, h=1024, 16 heads x 64) on 8 NeuronCores.

Sharding: tensor-parallel over heads. Core c owns heads {2c, 2c+1}:
  - qkv projection columns c*128:(c+1)*128 of each of Q/K/V blocks
  - w_out rows c*128:(c+1)*128
Each core computes a full [4096, 1024] partial of the output projection;
the host sums the 8 partials and adds the bias corrections.

Algebraic simplifications (exact up to float rounding):
  - k bias dropped: adds a per-query constant to logits -> softmax invariant.
  - v bias dropped in-kernel: contributes bv @ w_out (a constant row) to the
    output; added on the host together with b_out.
  - 1/sqrt(64) folded into wq/bq on the host.
  - softmax without max subtraction (|logits| <= ~2.1 for this distribution).

Per-core kernel (S^T scheme, feature-on-partition):
  xt = x^T in SBUF [128, 8, 4096] bf16 (hidden on partitions)
  Q^T, K^T per batch [128, 2048] bf16, then per-head row-duplicated so
  consecutive-kt S^T matmuls can row-tile onto disjoint PE row groups.
  V natural per batch [token, vcol] bf16 with a ones column per head.
  S^T tile [k 128, q 512] = K^T_h-slice x Q^T_h   (K=64, kt pairs row-tiled)
  P^T = exp(S^T) on ScalarE per kt-group of 2 (one ACTIVATE per [128, 1024])
  O^T_aug [65, 512] += V_aug-slices x P^T        (row 64 = softmax sums)
  epilogue per 128-q tile: PE-transpose O^T -> O, scale rows by 1/sum,
  transpose back, out [q 128, 512] = O_scaled^T x w_out, DVE evac, DMA out.

Scheduling: engines execute a static per-engine order, so the emission order
IS the schedule. The attention backbone is software-pipelined (S-pair and exp
of group g2, then AV of group g2-1, so the PE never in-order-stalls waiting
for exp), and a filler queue injects stage-A units of the other batch and
deferred epilogue units into the backbone's exp-wait bubbles.
"""

import contextlib
import sys
from collections import deque

import numpy as np

sys.path.insert(0, "/opt/trn_rl_repo")

import ml_dtypes  # noqa: E402

import concourse.bass as bass  # noqa: E402
import concourse.tile as tile  # noqa: E402
from concourse import bacc, mybir  # noqa: E402
from concourse.bass_utils import run_bass_kernel_spmd  # noqa: E402
from concourse.masks import make_identity  # noqa: E402

BF16 = mybir.dt.bfloat16
F32 = mybir.dt.float32
AF = mybir.ActivationFunctionType

B = 2
S = 2048
T = B * S          # 4096 tokens
H = 1024           # hidden
HD = 64            # head dim
N_CORES = 8

_program_cache = {}


class Ctx:
    pass


class Filler:
    """FIFO of generators; pull() advances the head generator one unit."""

    def __init__(self):
        self.q = deque()

    def add(self, gen):
        self.q.append(gen)

    def add_front(self, gen):
        self.q.appendleft(gen)

    def pull(self, n=1):
        while n > 0 and self.q:
            try:
                next(self.q[0])
                n -= 1
            except StopIteration:
                self.q.popleft()

    def drain(self):
        while self.q:
            self.pull()


def qk_units(nc, c, b):
    """Q^T/K^T projection for batch b in 512-token groups + head duplication."""
    for gl in range(4):
        g = b * 4 + gl
        sl = slice(g * 512, (g + 1) * 512)       # global token slice (for xt)
        ll = slice(gl * 512, (gl + 1) * 512)     # local token slice (per-batch)
        psq = c.psA.tile([128, 512], F32, tag="mm", name=f"psq{g}")
        for o in range(8):
            nc.tensor.matmul(
                psq[:], c.wq_sb[:, o, :], c.xt_sb[:, o, sl],
                start=(o == 0), stop=(o == 7),
            )
        nc.scalar.activation(c.QTs[b][:, ll], psq[:], AF.Identity, bias=c.bq_sb[:])
        for h in range(2):
            src = slice(h * 64, (h + 1) * 64)
            nc.vector.tensor_copy(c.QTd[b][h][0:64, ll], c.QTs[b][src, ll])
            nc.vector.tensor_copy(c.QTd[b][h][64:128, ll], c.QTs[b][src, ll])
        yield
        psk = c.psA.tile([128, 512], F32, tag="mm", name=f"psk{g}")
        for o in range(8):
            nc.tensor.matmul(
                psk[:], c.wk_sb[:, o, :], c.xt_sb[:, o, sl],
                start=(o == 0), stop=(o == 7),
            )
        nc.vector.tensor_copy(c.KTs[b][:, ll], psk[:])
        for h in range(2):
            src = slice(h * 64, (h + 1) * 64)
            nc.vector.tensor_copy(c.KTd[b][h][0:64, ll], c.KTs[b][src, ll])
            nc.vector.tensor_copy(c.KTd[b][h][64:128, ll], c.KTs[b][src, ll])
        yield


def v_units(nc, c, b):
    """V natural [token, vcol] for batch b; per token-tile layout:
    [0:64]=head0 V, 64=ones, [65:129]=head1 V, 129=ones."""
    for tl in range(16):
        t = b * 16 + tl
        psv = c.psA.tile([128, 512], F32, tag="mm", name=f"psv{t}")
        for o in range(8):
            nc.tensor.matmul(
                psv[:, 0:128], c.xt_sb[:, o, t * 128:(t + 1) * 128], c.wv_sb[:, o, :],
                start=(o == 0), stop=(o == 7),
            )
        # one strided copy fills both head halves (cols 0:64 and 65:129)
        nc.vector.tensor_copy(
            c.v_sb[b][:, tl, :].rearrange("p (g x) -> p g x", g=2)[:, :, 0:64],
            psv[:, 0:128].rearrange("p (g x) -> p g x", g=2),
        )
        yield


def epilogue_units(nc, c, b, qg, oT, use_act=False):
    """Normalize + output projection for one 512-query group (4 q-tiles).
    All scale phases (E1) first, then all projection phases (E2), so the
    PE->DVE->PE chains of different q-tiles overlap. use_act routes the PSUM
    evacuations to ScalarE (for the last group, whose epilogue runs in the
    kernel tail where ScalarE is otherwise idle)."""
    copy = nc.scalar.copy if use_act else (lambda o, i: nc.vector.tensor_copy(o, i))
    q0 = qg * 512
    gq0 = b * 2048 + q0
    onats = []
    for j in range(4):
        qj = q0 + j * 128
        ps_st = c.psA.tile([128, 512], F32, tag="mm", name=f"ps_st{b}{qg}{j}")
        nc.tensor.transpose(ps_st[:, 0:64], c.sums_sb[b][:, qj:qj + 128], c.ident64[:])
        recip = c.work.tile([128, 2], F32, tag="recip", name=f"recip{b}{qg}{j}")
        nc.vector.reciprocal(recip[:], ps_st[:, 0:33:32])
        pt1 = c.psA.tile([128, 128], BF16, tag="mm", name=f"pt1_{b}{qg}{j}")
        nc.tensor.transpose(pt1[:], oT[:, j * 128:(j + 1) * 128], c.ident[:])
        onat = c.work.tile([128, 128], BF16, tag=f"onat{j}", name=f"onat{b}{qg}{j}")
        nc.vector.tensor_scalar_mul(onat[:, 0:64], pt1[:, 0:64], recip[:, 0:1])
        nc.vector.tensor_scalar_mul(onat[:, 64:128], pt1[:, 64:128], recip[:, 1:2])
        onats.append(onat)
        yield
    if not use_act:
        for j in range(4):
            gqj = gq0 + j * 128
            pt2 = c.psA.tile([128, 128], BF16, tag="mm", name=f"pt2_{b}{qg}{j}")
            nc.tensor.transpose(pt2[:], onats[j][:], c.ident[:])
            osT = c.work.tile([128, 128], BF16, tag="osT", name=f"osT{b}{qg}{j}")
            copy(osT[:], pt2[:])
            for n in range(2):
                pso = c.psA.tile([128, 512], F32, tag="mm", name=f"pso{b}{qg}{j}{n}")
                nc.tensor.matmul(
                    pso[:], osT[:], c.wo_sb[:, n * 512:(n + 1) * 512],
                    start=True, stop=True,
                )
                ob = c.opool.tile([128, 512], F32, tag="ob", name=f"ob{b}{qg}{j}{n}")
                copy(ob[:], pso[:])
                nc.sync.dma_start(c.out[gqj:gqj + 128, n * 512:(n + 1) * 512], ob[:])
            yield
        return
    # Tail variant: this epilogue drains with nothing left to interleave, so
    # pipeline it explicitly — all transposes+copies first (copies alternating
    # between ScalarE and VectorE), then the projection matmuls with
    # alternating-engine evacuations.
    copies = [nc.scalar.copy, lambda o, i: nc.vector.tensor_copy(o, i)]
    osTs = []
    for j in range(4):
        pt2 = c.psA.tile([128, 128], BF16, tag="mm", name=f"pt2_{b}{qg}{j}")
        nc.tensor.transpose(pt2[:], onats[j][:], c.ident[:])
        osT = c.work.tile([128, 128], BF16, tag=f"osTt{j}", name=f"osT{b}{qg}{j}")
        copies[j % 2](osT[:], pt2[:])
        osTs.append(osT)
        if j % 2:
            yield
    for j in range(4):
        gqj = gq0 + j * 128
        for n in range(2):
            pso = c.psA.tile([128, 512], F32, tag="mm", name=f"pso{b}{qg}{j}{n}")
            nc.tensor.matmul(
                pso[:], osTs[j][:], c.wo_sb[:, n * 512:(n + 1) * 512],
                start=True, stop=True,
            )
            ob = c.opool.tile([128, 512], F32, tag="ob", name=f"ob{b}{qg}{j}{n}")
            copies[n](ob[:], pso[:])
            nc.sync.dma_start(c.out[gqj:gqj + 128, n * 512:(n + 1) * 512], ob[:])
        yield


def emit_s_exp(nc, c, b, qg, h, g2):
    """S^T row-tiled pair + exp for one kt-group; returns the P^T tile."""
    q0 = qg * 512
    ps2 = c.psS.tile([128, 2, 512], F32, tag="s2", name=f"ps2_{b}{qg}{h}{g2}")
    for j in range(2):
        kt = g2 * 2 + j
        rs = slice(j * 64, j * 64 + 64)
        k0 = kt * 128
        nc.tensor.matmul(
            ps2[:, j, :],
            c.KTd[b][h][rs, k0:k0 + 128],
            c.QTd[b][h][rs, q0:q0 + 512],
            start=True, stop=True,
            tile_position=(j * 64, 0),
        )
    pT = c.ptp.tile([128, 2, 512], BF16, tag="pT", name=f"pT{b}{qg}{h}{g2}")
    nc.scalar.activation(pT[:], ps2[:], AF.Exp)
    return pT


def emit_av(nc, c, b, qg, h, po, pT, g2):
    for j in range(2):
        kt = g2 * 2 + j
        nc.tensor.matmul(
            po[0:65, :],
            c.v_sb[b][:, kt, h * 65:h * 65 + 65],
            pT[:, j, :],
            start=(kt == 0), stop=(kt == 15),
        )


def emit_po_evac(nc, c, b, qg, h, po, oT):
    q0 = qg * 512
    nc.vector.tensor_copy(oT[h * 64:(h + 1) * 64, :], po[0:64, :])
    nc.vector.tensor_copy(c.sums_sb[b][h * 32:h * 32 + 1, q0:q0 + 512], po[64:65, :])


def warmup_qg0(nc, c, b, fill):
    """First query group of the first batch: run BOTH heads' S/exp chains while
    the filler emits this batch's QKV projections; all AVs are deferred into a
    filler generator so they interleave with the next query group's backbone."""
    pend = {0: deque(), 1: deque()}
    for g2 in range(8):
        for h in (0, 1):
            pT = emit_s_exp(nc, c, b, 0, h, g2)
            pend[h].append((pT, g2))
            fill.pull(2 if h == 0 else 1)
    oT = c.opool.tile([128, 512], BF16, tag="oT", name=f"oT{b}0")

    def av_burst():
        for h in (0, 1):
            po = c.psO.tile([128, 512], F32, tag="acc", name=f"po{b}0{h}")
            for pT, g2 in pend[h]:
                emit_av(nc, c, b, 0, h, po, pT, g2)
                yield
            emit_po_evac(nc, c, b, 0, h, po, oT)
            yield

    fill.add_front(av_burst())
    fill.add(epilogue_units(nc, c, b, 0, oT))


def stage_b(nc, c, b, fill, warmup_first):
    """Attention backbone for batch b, software-pipelined with filler units."""
    for qg in range(4):
        if warmup_first and qg == 0:
            warmup_qg0(nc, c, b, fill)
            continue
        oT = c.opool.tile([128, 512], BF16, tag="oT", name=f"oT{b}{qg}")
        for h in range(2):
            po = c.psO.tile([128, 512], F32, tag="acc", name=f"po{b}{qg}{h}")
            pend = deque()
            for g2 in range(8):
                pT = emit_s_exp(nc, c, b, qg, h, g2)
                if len(pend) >= 2:
                    emit_av(nc, c, b, qg, h, po, *pend.popleft())
                pend.append((pT, g2))
                fill.pull(1)
            while pend:
                emit_av(nc, c, b, qg, h, po, *pend.popleft())
            emit_po_evac(nc, c, b, qg, h, po, oT)
        fill.add(epilogue_units(nc, c, b, qg, oT, use_act=(b == 1 and qg == 3)))


def build_body(tc, xt, wq, wk, wv, bq, wo, out):
    nc = tc.nc
    c = Ctx()
    c.out = out
    with contextlib.ExitStack() as ctx:
        c.const = ctx.enter_context(tc.tile_pool(name="const", bufs=1))
        c.work = ctx.enter_context(tc.tile_pool(name="work", bufs=3))
        c.ptp = ctx.enter_context(tc.tile_pool(name="ptile", bufs=16))
        c.opool = ctx.enter_context(tc.tile_pool(name="opool", bufs=6))
        # PSUM budget (8 banks): s2 [128,2,512]f32 x2 bufs = 4, mm [128,512]f32
        # x2 bufs = 2, acc x2 = 2.
        c.psA = ctx.enter_context(tc.tile_pool(name="psA", bufs=2, space="PSUM"))
        c.psS = ctx.enter_context(tc.tile_pool(name="psS", bufs=2, space="PSUM"))
        c.psO = ctx.enter_context(tc.tile_pool(name="psO", bufs=2, space="PSUM"))

        # ---- DMA in consumption order: wq/bq, xt quarters 0-1, wk, wv,
        # xt quarters 2-3, wo ----
        c.wq_sb = c.const.tile([128, 8, 128], BF16, name="wq_sb")
        nc.sync.dma_start(c.wq_sb[:], wq[:])
        c.bq_sb = c.const.tile([128, 1], F32, name="bq_sb")
        nc.sync.dma_start(c.bq_sb[:], bq[:])
        actwarm = c.work.tile([1, 1], F32, tag="actwarm", name="actwarm")
        nc.scalar.activation(actwarm[:], c.bq_sb[0:1, 0:1], AF.Exp)

        c.xt_sb = c.const.tile([128, 8, T], BF16, name="xt_sb")
        xtr = xt.rearrange("(o p) t -> p o t", p=128)

        def load_xt(t0, t1):
            for o in range(8):
                nc.sync.dma_start(c.xt_sb[:, o, t0:t1], xtr[:, o, t0:t1])

        load_xt(0, 1024)
        c.wk_sb = c.const.tile([128, 8, 128], BF16, name="wk_sb")
        nc.sync.dma_start(c.wk_sb[:], wk[:])
        c.wv_sb = c.const.tile([128, 8, 128], BF16, name="wv_sb")
        nc.sync.dma_start(c.wv_sb[:], wv[:])
        load_xt(1024, 2048)
        load_xt(2048, 3072)
        load_xt(3072, 4096)
        c.wo_sb = c.const.tile([128, H], BF16, name="wo_sb")
        nc.sync.dma_start(c.wo_sb[:], wo[:])

        c.ident = c.const.tile([128, 128], BF16, name="ident")
        make_identity(nc, c.ident[:])
        c.ident64 = c.const.tile([64, 64], F32, name="ident64")
        make_identity(nc, c.ident64[:])

        # ---- per-batch tensors (disjoint, so batches schedule independently) ----
        c.QTs = [c.const.tile([128, S], BF16, name=f"QTs{b}") for b in range(2)]
        c.KTs = [c.const.tile([128, S], BF16, name=f"KTs{b}") for b in range(2)]
        c.QTd = [[c.const.tile([128, S], BF16, name=f"qtd{b}{h}") for h in range(2)]
                 for b in range(2)]
        c.KTd = [[c.const.tile([128, S], BF16, name=f"ktd{b}{h}") for h in range(2)]
                 for b in range(2)]
        c.v_sb = [c.const.tile([128, 16, 130], BF16, name=f"v_sb{b}") for b in range(2)]
        for b in range(2):
            nc.vector.memset(c.v_sb[b][:, :, 64:130:65], 1.0)
        # softmax sums land on partition 0 (head0) / 32 (head1)
        c.sums_sb = [c.const.tile([64, S], F32, name=f"sums_sb{b}") for b in range(2)]
        for b in range(2):
            nc.vector.memset(c.sums_sb[b][:], 0.0)

        # ---- emission ----
        fill = Filler()
        # First QK group of batch 0 up-front (the backbone needs Q/K group 0).
        qk0 = qk_units(nc, c, 0)
        next(qk0)
        next(qk0)
        fill.add(qk0)              # remaining 6 QK units of batch 0
        fill.add(v_units(nc, c, 0))
        fill.add(qk_units(nc, c, 1))
        fill.add(v_units(nc, c, 1))
        stage_b(nc, c, 0, fill, warmup_first=True)
        stage_b(nc, c, 1, fill, warmup_first=False)
        fill.drain()


def build_program():
    if "nc" in _program_cache:
        return _program_cache["nc"]
    nc = bacc.Bacc("TRN2", target_bir_lowering=False, debug=False)
    xt = nc.dram_tensor("xt", [H, T], BF16, kind="ExternalInput").ap()
    wq = nc.dram_tensor("wq", [128, 8, 128], BF16, kind="ExternalInput").ap()
    wk = nc.dram_tensor("wk", [128, 8, 128], BF16, kind="ExternalInput").ap()
    wv = nc.dram_tensor("wv", [128, 8, 128], BF16, kind="ExternalInput").ap()
    bq = nc.dram_tensor("bq", [128, 1], F32, kind="ExternalInput").ap()
    wo = nc.dram_tensor("wo", [128, H], BF16, kind="ExternalInput").ap()
    out = nc.dram_tensor("out", [T, H], F32, kind="ExternalOutput").ap()
    with tile.TileContext(nc) as tc:
        build_body(tc, xt, wq, wk, wv, bq, wo, out)
    nc.compile()
    _program_cache["nc"] = nc
    return nc


def make_in_maps(x, w_qkv, b_qkv, w_out):
    bf16 = ml_dtypes.bfloat16
    x = np.asarray(x, dtype=np.float32)
    w_qkv = np.asarray(w_qkv, dtype=np.float32)
    b_qkv = np.asarray(b_qkv, dtype=np.float32)
    w_out = np.asarray(w_out, dtype=np.float32)

    xt = np.ascontiguousarray(x.reshape(T, H).T).astype(bf16)  # [H, T]

    def prep_w(w):
        # [1024 hidden, 128] -> SBUF layout [128 part, 8 ktile, 128 col]
        return np.ascontiguousarray(w.reshape(8, 128, 128).transpose(1, 0, 2)).astype(bf16)

    in_maps = []
    for c in range(N_CORES):
        sl = slice(c * 128, (c + 1) * 128)
        in_maps.append({
            "xt": xt,
            "wq": prep_w(w_qkv[:, sl] * 0.125),
            "wk": prep_w(w_qkv[:, H + c * 128:H + (c + 1) * 128]),
            "wv": prep_w(w_qkv[:, 2 * H + c * 128:2 * H + (c + 1) * 128]),
            "bq": (b_qkv[sl] * 0.125).astype(np.float32).reshape(128, 1),
            "wo": np.ascontiguousarray(w_out[sl, :]).astype(bf16),
        })
    return in_maps


def finalize(results, b_qkv, b_out, w_out):
    b_qkv = np.asarray(b_qkv, dtype=np.float32)
    b_out = np.asarray(b_out, dtype=np.float32)
    w_out = np.asarray(w_out, dtype=np.float32)
    acc = np.zeros((T, H), np.float32)
    for r in results:
        acc += np.asarray(r["out"], dtype=np.float32)
    corr = b_out + b_qkv[2 * H:] @ w_out
    return (acc + corr).reshape(B, S, H).astype(np.float32)


def kernel(x, w_qkv, b_qkv, w_out, b_out):
    import os
    # NTFF tracing needs antenv.axon_hooks, which this client env lacks;
    # make sure an inherited BASS_TRACE can't route us into that path.
    os.environ["BASS_NEVER_TRACE"] = "1"
    nc = build_program()
    in_maps = make_in_maps(x, w_qkv, b_qkv, w_out)
    res = run_bass_kernel_spmd(nc, in_maps, list(range(N_CORES)))
    return finalize(res.results, b_qkv, b_out, w_out)

